# revision 9
# baseline (speedup 1.0000x reference)
"""Multi-head self-attention (RoPE + causal softmax) Bass kernel for TRN2.

Problem: B=2, H=16, S=2048, D_HEAD=64, fp32 I/O.
Sharding: 32 head-instances (B*H) split 4-per-core across 8 NeuronCores;
no cross-device communication.

Per-core kernel structure (4 heads, S=2048):
  - Q,K arrive host-pre-folded as head-pair tiles (128 partitions = s%128,
    free = [s_tile, headA_d | headB_d]).  RoPE is applied on DVE in this
    natural layout (pairing along the free dim), output in bf16.
  - XBAR DMA-transpose produces Q^T/K^T layouts (d on partitions, s on
    free), with two heads stacked on partitions 0-63 / 64-127.
  - Scores are computed transposed: S^T[k, q] = K^T.T @ Q^T per 128-row
    k-tile, causally trimmed to q >= k_tile_start, in 1024-column q-chunks.
  - exp(s/8) runs on ScalarE straight out of PSUM into bf16 SBUF (no-max
    softmax: scores are ~N(0,1) so exp never overflows).  Diagonal blocks
    get a 128x128 triangular mask via GPSIMD multiply.
  - V is shipped bf16 with a ones-column appended: out^T(65 x q) accumulates
    attn@[V|1] over k-tiles; row 64 is the softmax denominator.
  - 65x128 PE transposes + DVE reciprocal/scale produce the normalized
    (q, d) output tiles, DMA'd back to DRAM.
"""

import numpy as np
import ml_dtypes

import concourse.bass as bass
import concourse.tile as tile
from concourse import bacc, mybir
from concourse.bass_utils import run_bass_kernel_spmd

F32 = mybir.dt.float32
BF16 = mybir.dt.bfloat16
EXP = mybir.ActivationFunctionType.Exp

B, H, S_FULL, DH = 2, 16, 2048, 64
N_CORES = 8
HEADS_PER_CORE = (B * H) // N_CORES  # 4


# ---------------------------------------------------------------- device IR


def build_nc(n_heads=HEADS_PER_CORE, S=S_FULL, chunk=512, num_devices=N_CORES):
    """Build + compile the per-core Bass program (same program on all cores)."""
    NT = S // 128            # number of 128-row s-tiles
    npairs = n_heads // 2

    nc = bacc.Bacc(
        "TRN2", target_bir_lowering=False, debug=False, num_devices=num_devices
    )

    qp = nc.dram_tensor("qp", [npairs, 128, NT * 128], F32, kind="ExternalInput").ap()
    kp = nc.dram_tensor("kp", [npairs, 128, NT * 128], F32, kind="ExternalInput").ap()
    vx = nc.dram_tensor("vx", [n_heads, 128, NT * 65], BF16, kind="ExternalInput").ap()
    cosf = nc.dram_tensor("cosf", [128, NT * 128], F32, kind="ExternalInput").ap()
    sinf = nc.dram_tensor("sinf", [128, NT * 128], F32, kind="ExternalInput").ap()
    tri = nc.dram_tensor("tri", [128, 256], BF16, kind="ExternalInput").ap()
    ident = nc.dram_tensor("ident", [128, 128], F32, kind="ExternalInput").ap()
    o = nc.dram_tensor("o", [n_heads, 128, NT * DH], F32, kind="ExternalOutput").ap()

    with tile.TileContext(nc) as tc:
        _body(nc, tc, qp, kp, vx, cosf, sinf, tri, ident, o,
              n_heads=n_heads, S=S, chunk=chunk)

    nc.compile()
    return nc


def _body(nc, tc, qp, kp, vx, cosf, sinf, tri, ident, o, *, n_heads, S, chunk):
    from contextlib import ExitStack

    assert chunk == 512
    NT = S // 128
    npairs = n_heads // 2
    nchunks = S // chunk

    with ExitStack() as ctx:
        cpool = ctx.enter_context(tc.tile_pool(name="const", bufs=1))
        prep = ctx.enter_context(tc.tile_pool(name="prep", bufs=2))
        qkt = ctx.enter_context(tc.tile_pool(name="qkt", bufs=1))
        expp = ctx.enter_context(tc.tile_pool(name="expp", bufs=3))
        normp = ctx.enter_context(tc.tile_pool(name="normp", bufs=4))
        outp = ctx.enter_context(tc.tile_pool(name="outp", bufs=3))
        obuf = ctx.enter_context(tc.tile_pool(name="obuf", bufs=4))
        ps_s = ctx.enter_context(tc.tile_pool(name="ps_s", bufs=2, space="PSUM"))
        ps_oa = ctx.enter_context(tc.tile_pool(name="ps_oa", bufs=1, space="PSUM"))
        ps_ob = ctx.enter_context(tc.tile_pool(name="ps_ob", bufs=1, space="PSUM"))
        ps_t = ctx.enter_context(tc.tile_pool(name="ps_t", bufs=2, space="PSUM"))

        # ---- constants (v tiles loaded after the first prep chains)
        cos_t = cpool.tile([128, NT * 128], F32, tag="cos")
        nc.sync.dma_start(cos_t[:], cosf[:])
        sin_t = cpool.tile([128, NT * 128], F32, tag="sin")
        nc.sync.dma_start(sin_t[:], sinf[:])
        tri_t = cpool.tile([128, 256], BF16, tag="tri")
        nc.gpsimd.dma_start(tri_t[:], tri[:])
        id_t = cpool.tile([128, 128], F32, tag="id")
        nc.gpsimd.dma_start(id_t[:], ident[:])

        # ---- RoPE + transpose prep: build Q^T / K^T (two heads stacked).
        # K chains first so PE warm-up matmuls (below) can start against kT
        # while qT transposes finish; XBAR transposes alternate between the
        # two HWDGE issuing engines (sync / scalar) so they run in parallel.
        qT = [qkt.tile([128, NT * 128], BF16, tag=f"qT{p}", name=f"qT{p}")
              for p in range(npairs)]
        kT = [qkt.tile([128, NT * 128], BF16, tag=f"kT{p}", name=f"kT{p}")
              for p in range(npairs)]

        chains = []
        for pr in range(npairs):
            chains.append((kp[pr], kT[pr]))
            chains.append((qp[pr], qT[pr]))
        HT = NT // 2
        for ci, (src_ap, dstT) in enumerate(chains):
            nat = prep.tile([128, NT * 128], F32, tag="nat")
            (nc.sync if ci < 2 else nc.gpsimd).dma_start(nat[:], src_ap)
            n3 = nat[:].rearrange("p (t d) -> p t d", d=128)
            s3 = sin_t[:].rearrange("p (t d) -> p t d", d=128)
            c3 = cos_t[:].rearrange("p (t d) -> p t d", d=128)
            t1 = prep.tile([128, NT * 128], F32, tag="t1")
            t2 = prep.tile([128, NT * 128], F32, tag="t2")
            t13 = t1[:].rearrange("p (t d) -> p t d", d=128)
            t23 = t2[:].rearrange("p (t d) -> p t d", d=128)
            ro = prep.tile([128, NT * 128], BF16, tag="ro")
            r3 = ro[:].rearrange("p (t d) -> p t d", d=128)
            # RoPE in half-tensor slices so XBAR transposes of the first
            # half overlap DVE work on the second half:
            # t1 = x * cos ; t2_even = x_odd * (-sin)_even ;
            # t2_odd = x_even * sin_odd ; out = t1 + t2   (bf16)
            for hf in range(2):
                ts = slice(hf * HT, (hf + 1) * HT)
                nc.vector.tensor_mul(t13[:, ts], n3[:, ts], c3[:, ts])
                nc.vector.tensor_mul(
                    t23[:, ts, 0::2], n3[:, ts, 1::2], s3[:, ts, 0::2])
                nc.vector.tensor_mul(
                    t23[:, ts, 1::2], n3[:, ts, 0::2], s3[:, ts, 1::2])
                nc.vector.tensor_add(r3[:, ts], t13[:, ts], t23[:, ts])
                for t in range(hf * HT, (hf + 1) * HT):
                    # pair-0 chains split across both HWDGE issuers for
                    # latency; later chains go to sync only (scalar is busy
                    # with exp by then)
                    if ci < 2:
                        eng = nc.sync if t % 2 == 0 else nc.scalar
                    else:
                        eng = nc.scalar if t % 4 == 1 else nc.sync
                    eng.dma_start_transpose(
                        dstT[:, t * 128:(t + 1) * 128], r3[:, t, :]
                    )

        vts = []
        for h in range(n_heads):
            vt = cpool.tile([128, NT * 65], BF16, tag=f"v{h}", name=f"v{h}")
            nc.gpsimd.dma_start(vt[:], vx[h])
            vts.append(vt)

        # ---- PE warm-up: dummy matmuls against kT[0] (ready before qT[0])
        # so the HAM clock gate reaches 8/8 before the first real matmul.
        s_d = ps_s.tile([128, 1024], F32, tag="s")
        for i in range(24):
            nc.tensor.matmul(
                s_d[:, 0:512], kT[0][0:64, 0:128], kT[0][0:64, 0:512],
                start=True, stop=True,
            )

        # ---- scores / softmax / attn@V: head pairs, 512-wide q-chunks.
        # Head A (partitions 0:64) and head B (64:128) issue adjacent
        # matmul1s into different PSUM banks -> concurrent row-group
        # execution in the PE array.  One ACTIVATE covers both heads'
        # scores via a strided (128, 2, 512-rel) access pattern.
        pending_norm = []

        def flush_norm():
            while pending_norm:
                pending_norm.pop(0)()

        for pr in range(npairs):
            hA, hB = 2 * pr, 2 * pr + 1
            qA, kA = qT[pr][0:64, :], kT[pr][0:64, :]
            qB, kB = qT[pr][64:128, :], kT[pr][64:128, :]
            v3A = vts[hA][:].rearrange("p (t j) -> p t j", j=65)
            v3B = vts[hB][:].rearrange("p (t j) -> p t j", j=65)
            obA = obuf.tile([128, NT * DH], F32, tag="ob", name="obA")
            obB = obuf.tile([128, NT * DH], F32, tag="ob", name="obB")
            for qc in range(nchunks):
                q0 = qc * chunk
                kpc = chunk // 128
                outA = ps_oa.tile([65, 512], F32, tag="outa")
                outB = ps_ob.tile([65, 512], F32, tag="outb")
                ktmax = (qc + 1) * kpc

                stage = []   # 1-round-deferred exp/mask/mm2
                for kt in range(ktmax):
                    rel = max(128 * kt, q0) - q0
                    s_t = ps_s.tile([128, 1024], F32, tag="s")
                    nc.tensor.matmul(
                        s_t[:, rel:512],
                        kA[:, kt * 128:(kt + 1) * 128],
                        qA[:, q0 + rel:q0 + 512],
                        start=True, stop=True,
                    )
                    nc.tensor.matmul(
                        s_t[:, 512 + rel:1024],
                        kB[:, kt * 128:(kt + 1) * 128],
                        qB[:, q0 + rel:q0 + 512],
                        start=True, stop=True,
                    )

                    def consume(kt=kt, rel=rel, s_t=s_t):
                        diag = kt >= qc * kpc
                        last = kt == ktmax - 1
                        s3v = s_t[:].rearrange("p (x q) -> p x q", x=2)
                        ex = expp.tile([128, 1024], BF16, tag="ex")
                        e3 = ex[:].rearrange("p (x q) -> p x q", x=2)
                        nc.scalar.activation(
                            e3[:, :, rel:], s3v[:, :, rel:], EXP, scale=0.125
                        )
                        if diag:
                            # causal mask on the diagonal 128-col block
                            nc.vector.tensor_mul(
                                e3[:, :, rel:rel + 128],
                                e3[:, :, rel:rel + 128],
                                tri_t[:].rearrange("p (x q) -> p x q", x=2),
                            )
                        for half, (out_t, v3) in enumerate(
                            ((outA, v3A), (outB, v3B))
                        ):
                            nc.tensor.matmul(
                                out_t[:, rel:512],
                                v3[:, kt, :],
                                ex[:, 512 * half + rel:512 * half + 512],
                                start=(kt == 0), stop=last,
                            )

                    stage.append(consume)
                    if len(stage) > 1:
                        stage.pop(0)()
                while stage:
                    stage.pop(0)()

                # copy the accumulated chunks out of PSUM promptly
                soA = normp.tile([65, 512], F32, tag="so", name="soA")
                nc.vector.tensor_copy(soA[:], outA[:])
                soB = normp.tile([65, 512], F32, tag="so", name="soB")
                nc.vector.tensor_copy(soB[:], outB[:])

                def norm(qc=qc, soA=soA, soB=soB, obA=obA, obB=obB):
                    for so, ob in ((soA, obA), (soB, obB)):
                        for j in range(chunk // 128):
                            tr = ps_t.tile([128, 65], F32, tag="tr")
                            nc.tensor.transpose(
                                tr[:], so[:, j * 128:(j + 1) * 128],
                                id_t[0:65, 0:65]
                            )
                            rc = outp.tile([128, 1], F32, tag="rc")
                            nc.vector.reciprocal(rc[:], tr[:, 64:65])
                            jj = qc * (chunk // 128) + j
                            nc.vector.tensor_scalar_mul(
                                ob[:, jj * DH:(jj + 1) * DH], tr[:, 0:DH], rc[:]
                            )

                # run the *previous* chunk's normalization now (its deps are
                # long satisfied) so PE isn't stalled right at chunk start
                flush_norm()
                pending_norm.append(norm)
            flush_norm()
            nc.gpsimd.dma_start(o[hA], obA[:])
            nc.gpsimd.dma_start(o[hB], obB[:])
        flush_norm()


# ---------------------------------------------------------------- host side


def _rope_tables(S):
    position = np.arange(S, dtype=np.float32)[:, None]
    div = (np.float32(10000.0)
           ** (np.arange(0, DH, 2, dtype=np.float32) / np.float32(DH)))
    div = np.repeat(div.astype(np.float32), 2)
    ang = position / div[None, :]
    cos = np.cos(ang).astype(np.float32)
    sin = np.sin(ang).astype(np.float32)
    sgn = np.ones(DH, np.float32)
    sgn[0::2] = -1.0
    return cos, sin * sgn


def _fold(tab, S):
    # (S, DH) -> (128, NT, DH): [p, t, d] = tab[t*128 + p, d]
    NT = S // 128
    return np.ascontiguousarray(tab.reshape(NT, 128, DH).transpose(1, 0, 2))


def host_inputs(qh, kh, vh, S):
    """Per-core input prep.  qh/kh/vh: (n_heads, S, DH) fp32."""
    n_heads = qh.shape[0]
    NT = S // 128
    npairs = n_heads // 2

    def pack_pairs(x):
        # (n_heads, S, DH) -> (npairs, 128, NT*128), two heads interleaved
        a = x.reshape(npairs, 2, NT, 128, DH).transpose(0, 3, 2, 1, 4)
        return np.ascontiguousarray(a.reshape(npairs, 128, NT * 128))

    vt = vh.reshape(n_heads, NT, 128, DH).transpose(0, 2, 1, 3)  # (h,128,NT,DH)
    vext = np.concatenate(
        [vt, np.ones((n_heads, 128, NT, 1), np.float32)], axis=3
    ).astype(ml_dtypes.bfloat16)

    cos, sinS = _rope_tables(S)
    cosf = _fold(cos, S)
    sinf = _fold(sinS, S)
    # duplicate along d for the two stacked heads -> (128, NT, 128)
    cosf2 = np.concatenate([cosf, cosf], axis=2)
    sinf2 = np.concatenate([sinf, sinf], axis=2)

    tri1 = np.triu(np.ones((128, 128), np.float32))
    tri = np.concatenate([tri1, tri1], axis=1).astype(ml_dtypes.bfloat16)
    ident = np.eye(128, dtype=np.float32)

    return {
        "qp": pack_pairs(qh),
        "kp": pack_pairs(kh),
        "vx": np.ascontiguousarray(vext.reshape(n_heads, 128, NT * 65)),
        "cosf": np.ascontiguousarray(cosf2.reshape(128, NT * 128)),
        "sinf": np.ascontiguousarray(sinf2.reshape(128, NT * 128)),
        "tri": tri,
        "ident": ident,
    }


_NC_CACHE = {}


def _get_nc():
    if "nc" not in _NC_CACHE:
        _NC_CACHE["nc"] = build_nc()
    return _NC_CACHE["nc"]


def kernel(q, k, v):
    q = np.asarray(q)
    k = np.asarray(k)
    v = np.asarray(v)
    nc = _get_nc()

    # faithful raw-view head split (matches torch .view semantics)
    qh = q.reshape(B * H, S_FULL, DH)
    kh = k.reshape(B * H, S_FULL, DH)
    vh = v.reshape(B * H, S_FULL, DH)

    in_maps = []
    for c in range(N_CORES):
        sl = slice(c * HEADS_PER_CORE, (c + 1) * HEADS_PER_CORE)
        in_maps.append(host_inputs(qh[sl], kh[sl], vh[sl], S_FULL))

    res = run_bass_kernel_spmd(nc, in_maps, list(range(N_CORES)))

    NT = S_FULL // 128
    out = np.empty((B * H, S_FULL, DH), np.float32)
    for c in range(N_CORES):
        oc = res.results[c]["o"]  # (heads, 128, NT*DH)
        oc = oc.reshape(HEADS_PER_CORE, 128, NT, DH).transpose(0, 2, 1, 3)
        out[c * HEADS_PER_CORE:(c + 1) * HEADS_PER_CORE] = oc.reshape(
            HEADS_PER_CORE, S_FULL, DH
        )
    return out.reshape(B, S_FULL, H * DH)


# revision 10
# speedup vs baseline: 1.0487x; 1.0487x over previous
"""Multi-head self-attention (RoPE + causal softmax) Bass kernel for TRN2.

Problem: B=2, H=16, S=2048, D_HEAD=64, fp32 I/O.
Sharding: 32 head-instances (B*H) split 4-per-core across 8 NeuronCores;
no cross-device communication.

Per-core kernel structure (4 heads, S=2048):
  - Q,K arrive host-pre-folded as head-pair tiles (128 partitions = s%128,
    free = [s_tile, headA_d | headB_d]).  RoPE is applied on DVE in this
    natural layout (pairing along the free dim), output in bf16.
  - XBAR DMA-transpose produces Q^T/K^T layouts (d on partitions, s on
    free), with two heads stacked on partitions 0-63 / 64-127.
  - Scores are computed transposed: S^T[k, q] = K^T.T @ Q^T per 128-row
    k-tile, causally trimmed to q >= k_tile_start, in 1024-column q-chunks.
  - exp(s/8) runs on ScalarE straight out of PSUM into bf16 SBUF (no-max
    softmax: scores are ~N(0,1) so exp never overflows).  Diagonal blocks
    get a 128x128 triangular mask via GPSIMD multiply.
  - V is shipped bf16 with a ones-column appended: out^T(65 x q) accumulates
    attn@[V|1] over k-tiles; row 64 is the softmax denominator.
  - 65x128 PE transposes + DVE reciprocal/scale produce the normalized
    (q, d) output tiles, DMA'd back to DRAM.
"""

import numpy as np
import ml_dtypes

import concourse.bass as bass
import concourse.tile as tile
from concourse import bacc, mybir
from concourse.bass_utils import run_bass_kernel_spmd

F32 = mybir.dt.float32
BF16 = mybir.dt.bfloat16
EXP = mybir.ActivationFunctionType.Exp

B, H, S_FULL, DH = 2, 16, 2048, 64
N_CORES = 8
HEADS_PER_CORE = (B * H) // N_CORES  # 4


# ---------------------------------------------------------------- device IR


def build_nc(n_heads=HEADS_PER_CORE, S=S_FULL, chunk=512, num_devices=N_CORES):
    """Build + compile the per-core Bass program (same program on all cores)."""
    NT = S // 128            # number of 128-row s-tiles
    npairs = n_heads // 2

    nc = bacc.Bacc(
        "TRN2", target_bir_lowering=False, debug=False, num_devices=num_devices
    )

    qp = nc.dram_tensor("qp", [npairs, 128, NT * 128], F32, kind="ExternalInput").ap()
    kp = nc.dram_tensor("kp", [npairs, 128, NT * 128], F32, kind="ExternalInput").ap()
    vx = nc.dram_tensor("vx", [n_heads, 128, NT * 65], BF16, kind="ExternalInput").ap()
    cosf = nc.dram_tensor("cosf", [128, NT * 128], F32, kind="ExternalInput").ap()
    sinf = nc.dram_tensor("sinf", [128, NT * 128], F32, kind="ExternalInput").ap()
    tri = nc.dram_tensor("tri", [128, 256], BF16, kind="ExternalInput").ap()
    ident = nc.dram_tensor("ident", [128, 128], F32, kind="ExternalInput").ap()
    o = nc.dram_tensor("o", [n_heads, 128, NT * DH], F32, kind="ExternalOutput").ap()

    with tile.TileContext(nc) as tc:
        _body(nc, tc, qp, kp, vx, cosf, sinf, tri, ident, o,
              n_heads=n_heads, S=S, chunk=chunk)

    nc.compile()
    return nc


def _body(nc, tc, qp, kp, vx, cosf, sinf, tri, ident, o, *, n_heads, S, chunk):
    from contextlib import ExitStack

    assert chunk == 512
    NT = S // 128
    npairs = n_heads // 2
    nchunks = S // chunk

    with ExitStack() as ctx:
        cpool = ctx.enter_context(tc.tile_pool(name="const", bufs=1))
        prep = ctx.enter_context(tc.tile_pool(name="prep", bufs=2))
        qkt = ctx.enter_context(tc.tile_pool(name="qkt", bufs=1))
        expp = ctx.enter_context(tc.tile_pool(name="expp", bufs=3))
        normp = ctx.enter_context(tc.tile_pool(name="normp", bufs=4))
        outp = ctx.enter_context(tc.tile_pool(name="outp", bufs=3))
        obuf = ctx.enter_context(tc.tile_pool(name="obuf", bufs=4))
        ps_s = ctx.enter_context(tc.tile_pool(name="ps_s", bufs=2, space="PSUM"))
        ps_oa = ctx.enter_context(tc.tile_pool(name="ps_oa", bufs=1, space="PSUM"))
        ps_ob = ctx.enter_context(tc.tile_pool(name="ps_ob", bufs=1, space="PSUM"))
        ps_t = ctx.enter_context(tc.tile_pool(name="ps_t", bufs=2, space="PSUM"))

        # ---- constants (v tiles loaded after the first prep chains)
        cos_t = cpool.tile([128, NT * 128], F32, tag="cos")
        nc.sync.dma_start(cos_t[:], cosf[:])
        sin_t = cpool.tile([128, NT * 128], F32, tag="sin")
        nc.sync.dma_start(sin_t[:], sinf[:])
        tri_t = cpool.tile([128, 256], BF16, tag="tri")
        nc.gpsimd.dma_start(tri_t[:], tri[:])
        id_t = cpool.tile([128, 128], F32, tag="id")
        nc.gpsimd.dma_start(id_t[:], ident[:])

        # ---- RoPE + transpose prep: build Q^T / K^T (two heads stacked).
        # K chains first so PE warm-up matmuls (below) can start against kT
        # while qT transposes finish; XBAR transposes alternate between the
        # two HWDGE issuing engines (sync / scalar) so they run in parallel.
        qT = [qkt.tile([128, NT * 128], BF16, tag=f"qT{p}", name=f"qT{p}")
              for p in range(npairs)]
        kT = [qkt.tile([128, NT * 128], BF16, tag=f"kT{p}", name=f"kT{p}")
              for p in range(npairs)]

        chains = []
        for pr in range(npairs):
            chains.append((kp[pr], kT[pr]))
            chains.append((qp[pr], qT[pr]))
        HT = NT // 2
        for ci, (src_ap, dstT) in enumerate(chains):
            nat = prep.tile([128, NT * 128], F32, tag="nat")
            (nc.sync if ci < 2 else nc.gpsimd).dma_start(nat[:], src_ap)
            n3 = nat[:].rearrange("p (t d) -> p t d", d=128)
            s3 = sin_t[:].rearrange("p (t d) -> p t d", d=128)
            c3 = cos_t[:].rearrange("p (t d) -> p t d", d=128)
            t1 = prep.tile([128, NT * 128], F32, tag="t1")
            t2 = prep.tile([128, NT * 128], F32, tag="t2")
            t13 = t1[:].rearrange("p (t d) -> p t d", d=128)
            t23 = t2[:].rearrange("p (t d) -> p t d", d=128)
            ro = prep.tile([128, NT * 128], BF16, tag="ro")
            r3 = ro[:].rearrange("p (t d) -> p t d", d=128)
            # RoPE in half-tensor slices so XBAR transposes of the first
            # half overlap DVE work on the second half:
            # t1 = x * cos ; t2_even = x_odd * (-sin)_even ;
            # t2_odd = x_even * sin_odd ; out = t1 + t2   (bf16)
            for hf in range(2):
                ts = slice(hf * HT, (hf + 1) * HT)
                nc.vector.tensor_mul(t13[:, ts], n3[:, ts], c3[:, ts])
                nc.vector.tensor_mul(
                    t23[:, ts, 0::2], n3[:, ts, 1::2], s3[:, ts, 0::2])
                nc.vector.tensor_mul(
                    t23[:, ts, 1::2], n3[:, ts, 0::2], s3[:, ts, 1::2])
                nc.vector.tensor_add(r3[:, ts], t13[:, ts], t23[:, ts])
                for t in range(hf * HT, (hf + 1) * HT):
                    # pair-0 chains split across both HWDGE issuers for
                    # latency; later chains go to sync only (scalar is busy
                    # with exp by then)
                    if ci < 2:
                        eng = nc.sync if t % 2 == 0 else nc.scalar
                    else:
                        eng = nc.sync
                    eng.dma_start_transpose(
                        dstT[:, t * 128:(t + 1) * 128], r3[:, t, :]
                    )

        vts = []
        for h in range(n_heads):
            vt = cpool.tile([128, NT * 65], BF16, tag=f"v{h}", name=f"v{h}")
            nc.gpsimd.dma_start(vt[:], vx[h])
            vts.append(vt)

        # ---- PE warm-up: dummy matmuls against kT[0] (ready before qT[0])
        # so the HAM clock gate reaches 8/8 before the first real matmul.
        s_d = ps_s.tile([128, 1024], F32, tag="s")
        for i in range(16):
            nc.tensor.matmul(
                s_d[:, 0:128], kT[0][0:64, 0:128], kT[0][0:64, 0:128],
                start=True, stop=True,
            )

        # ---- scores / softmax / attn@V: head pairs, 512-wide q-chunks.
        # Head A (partitions 0:64) and head B (64:128) issue adjacent
        # matmul1s into different PSUM banks -> concurrent row-group
        # execution in the PE array.  One ACTIVATE covers both heads'
        # scores via a strided (128, 2, 512-rel) access pattern.
        pending_norm = []

        def flush_norm():
            while pending_norm:
                pending_norm.pop(0)()

        for pr in range(npairs):
            hA, hB = 2 * pr, 2 * pr + 1
            qA, kA = qT[pr][0:64, :], kT[pr][0:64, :]
            qB, kB = qT[pr][64:128, :], kT[pr][64:128, :]
            v3A = vts[hA][:].rearrange("p (t j) -> p t j", j=65)
            v3B = vts[hB][:].rearrange("p (t j) -> p t j", j=65)
            obA = obuf.tile([128, NT * DH], F32, tag="ob", name="obA")
            obB = obuf.tile([128, NT * DH], F32, tag="ob", name="obB")
            for qc in range(nchunks):
                q0 = qc * chunk
                kpc = chunk // 128
                outA = ps_oa.tile([65, 512], F32, tag="outa")
                outB = ps_ob.tile([65, 512], F32, tag="outb")
                ktmax = (qc + 1) * kpc

                stage = []   # 1-round-deferred exp/mask/mm2
                for kt in range(ktmax):
                    rel = max(128 * kt, q0) - q0
                    s_t = ps_s.tile([128, 1024], F32, tag="s")
                    nc.tensor.matmul(
                        s_t[:, rel:512],
                        kA[:, kt * 128:(kt + 1) * 128],
                        qA[:, q0 + rel:q0 + 512],
                        start=True, stop=True,
                    )
                    nc.tensor.matmul(
                        s_t[:, 512 + rel:1024],
                        kB[:, kt * 128:(kt + 1) * 128],
                        qB[:, q0 + rel:q0 + 512],
                        start=True, stop=True,
                    )

                    def consume(kt=kt, rel=rel, s_t=s_t):
                        diag = kt >= qc * kpc
                        last = kt == ktmax - 1
                        s3v = s_t[:].rearrange("p (x q) -> p x q", x=2)
                        ex = expp.tile([128, 1024], BF16, tag="ex")
                        e3 = ex[:].rearrange("p (x q) -> p x q", x=2)
                        nc.scalar.activation(
                            e3[:, :, rel:], s3v[:, :, rel:], EXP, scale=0.125
                        )
                        if diag:
                            # causal mask on the diagonal 128-col block
                            nc.vector.tensor_mul(
                                e3[:, :, rel:rel + 128],
                                e3[:, :, rel:rel + 128],
                                tri_t[:].rearrange("p (x q) -> p x q", x=2),
                            )
                        for half, (out_t, v3) in enumerate(
                            ((outA, v3A), (outB, v3B))
                        ):
                            nc.tensor.matmul(
                                out_t[:, rel:512],
                                v3[:, kt, :],
                                ex[:, 512 * half + rel:512 * half + 512],
                                start=(kt == 0), stop=last,
                            )

                    stage.append(consume)
                    if len(stage) > 1:
                        stage.pop(0)()
                while stage:
                    stage.pop(0)()

                # copy the accumulated chunks out of PSUM promptly
                soA = normp.tile([65, 512], F32, tag="so", name="soA")
                nc.vector.tensor_copy(soA[:], outA[:])
                soB = normp.tile([65, 512], F32, tag="so", name="soB")
                nc.vector.tensor_copy(soB[:], outB[:])

                def norm(qc=qc, soA=soA, soB=soB, obA=obA, obB=obB):
                    for so, ob in ((soA, obA), (soB, obB)):
                        for j in range(chunk // 128):
                            tr = ps_t.tile([128, 65], F32, tag="tr")
                            nc.tensor.transpose(
                                tr[:], so[:, j * 128:(j + 1) * 128],
                                id_t[0:65, 0:65]
                            )
                            rc = outp.tile([128, 1], F32, tag="rc")
                            nc.vector.reciprocal(rc[:], tr[:, 64:65])
                            jj = qc * (chunk // 128) + j
                            nc.vector.tensor_scalar_mul(
                                ob[:, jj * DH:(jj + 1) * DH], tr[:, 0:DH], rc[:]
                            )

                # run the *previous* chunk's normalization now (its deps are
                # long satisfied) so PE isn't stalled right at chunk start
                flush_norm()
                pending_norm.append(norm)
            flush_norm()
            nc.gpsimd.dma_start(o[hA], obA[:])
            nc.gpsimd.dma_start(o[hB], obB[:])
        flush_norm()


# ---------------------------------------------------------------- host side


def _rope_tables(S):
    position = np.arange(S, dtype=np.float32)[:, None]
    div = (np.float32(10000.0)
           ** (np.arange(0, DH, 2, dtype=np.float32) / np.float32(DH)))
    div = np.repeat(div.astype(np.float32), 2)
    ang = position / div[None, :]
    cos = np.cos(ang).astype(np.float32)
    sin = np.sin(ang).astype(np.float32)
    sgn = np.ones(DH, np.float32)
    sgn[0::2] = -1.0
    return cos, sin * sgn


def _fold(tab, S):
    # (S, DH) -> (128, NT, DH): [p, t, d] = tab[t*128 + p, d]
    NT = S // 128
    return np.ascontiguousarray(tab.reshape(NT, 128, DH).transpose(1, 0, 2))


def host_inputs(qh, kh, vh, S):
    """Per-core input prep.  qh/kh/vh: (n_heads, S, DH) fp32."""
    n_heads = qh.shape[0]
    NT = S // 128
    npairs = n_heads // 2

    def pack_pairs(x):
        # (n_heads, S, DH) -> (npairs, 128, NT*128), two heads interleaved
        a = x.reshape(npairs, 2, NT, 128, DH).transpose(0, 3, 2, 1, 4)
        return np.ascontiguousarray(a.reshape(npairs, 128, NT * 128))

    vt = vh.reshape(n_heads, NT, 128, DH).transpose(0, 2, 1, 3)  # (h,128,NT,DH)
    vext = np.concatenate(
        [vt, np.ones((n_heads, 128, NT, 1), np.float32)], axis=3
    ).astype(ml_dtypes.bfloat16)

    cos, sinS = _rope_tables(S)
    cosf = _fold(cos, S)
    sinf = _fold(sinS, S)
    # duplicate along d for the two stacked heads -> (128, NT, 128)
    cosf2 = np.concatenate([cosf, cosf], axis=2)
    sinf2 = np.concatenate([sinf, sinf], axis=2)

    tri1 = np.triu(np.ones((128, 128), np.float32))
    tri = np.concatenate([tri1, tri1], axis=1).astype(ml_dtypes.bfloat16)
    ident = np.eye(128, dtype=np.float32)

    return {
        "qp": pack_pairs(qh),
        "kp": pack_pairs(kh),
        "vx": np.ascontiguousarray(vext.reshape(n_heads, 128, NT * 65)),
        "cosf": np.ascontiguousarray(cosf2.reshape(128, NT * 128)),
        "sinf": np.ascontiguousarray(sinf2.reshape(128, NT * 128)),
        "tri": tri,
        "ident": ident,
    }


_NC_CACHE = {}


def _get_nc():
    if "nc" not in _NC_CACHE:
        _NC_CACHE["nc"] = build_nc()
    return _NC_CACHE["nc"]


def kernel(q, k, v):
    q = np.asarray(q)
    k = np.asarray(k)
    v = np.asarray(v)
    nc = _get_nc()

    # faithful raw-view head split (matches torch .view semantics)
    qh = q.reshape(B * H, S_FULL, DH)
    kh = k.reshape(B * H, S_FULL, DH)
    vh = v.reshape(B * H, S_FULL, DH)

    in_maps = []
    for c in range(N_CORES):
        sl = slice(c * HEADS_PER_CORE, (c + 1) * HEADS_PER_CORE)
        in_maps.append(host_inputs(qh[sl], kh[sl], vh[sl], S_FULL))

    res = run_bass_kernel_spmd(nc, in_maps, list(range(N_CORES)))

    NT = S_FULL // 128
    out = np.empty((B * H, S_FULL, DH), np.float32)
    for c in range(N_CORES):
        oc = res.results[c]["o"]  # (heads, 128, NT*DH)
        oc = oc.reshape(HEADS_PER_CORE, 128, NT, DH).transpose(0, 2, 1, 3)
        out[c * HEADS_PER_CORE:(c + 1) * HEADS_PER_CORE] = oc.reshape(
            HEADS_PER_CORE, S_FULL, DH
        )
    return out.reshape(B, S_FULL, H * DH)
